# revision 32
# baseline (speedup 1.0000x reference)
"""AttnConv2d Trainium2 kernel.

Per-core = one batch image (data-parallel over 8 NeuronCores), with a
2-scalar AllReduce for the global attn mean/std.

Pipeline per core:
  1. key = conv3x3(x1, key_w), query = conv3x3(x2, query_w), computed in
     "transposed" orientation (image shifts stationary, weights moving) so
     the conv output lands pixel-major [pix, ch], grouped by (y%3, x%3)
     residue class -- exactly the layout the attention contraction needs.
     To satisfy the walrus rule that a matmul stationary AP has ONE free
     dim, the host pre-splits each padded image into 9 residue-class
     planes on a 33x33 patch grid; shifted conv reads are then contiguous
     128-element runs. Patch-grid edge lanes are garbage and get zeroed
     by a 0/1 mask on the PSUM->SBUF copy of keyT.
  2. attn[k, ci, co] = sum_pix key[pix, ci] * query[pix, co] per class k.
  3. global mean/std over all B*Cout*Cin*9 attn values (AllReduce of
     sum/sumsq), attn' = km*attn + (attn - m)/(s + eps).
  4. out = conv3x3(x1, attn') in standard orientation (attn stationary),
     giving NCHW fp32 output directly.
"""
import os
import sys

for _p in ("/opt/trn_rl_repo",):
    if os.path.isdir(_p) and _p not in sys.path:
        sys.path.append(_p)

import ml_dtypes
import numpy as np

import concourse.bass as bass
import concourse.bacc as bacc
import concourse.tile as tile
from concourse import mybir
from concourse.bass_utils import run_bass_kernel_spmd

F32 = mybir.dt.float32
BF16 = mybir.dt.bfloat16

N_CORES = 8
B, CIN, COUT, H, W = 8, 192, 192, 96, 96
KS, FK, PAD = 3, 9, 1
HP = H + 2  # 98, padded image
NPP = HP * HP  # 9604
HW = H * W  # 9216
C0, C1 = 128, 64  # channel chunks (192 = 128 + 64)
EPS = 1e-4
NTOT = float(B * COUT * CIN * FK)  # 2654208 values in global stats

# residue-class patch planes: 33x33 patch grid per class, flat stride 33
PQ = 33
PLANE_VALID = PQ * 32  # flat f < 1056 covers all valid patch rows
NCHK = 9  # ceil(1056 / 128) pixel chunks per class
PLANE_STRIDE = 1216  # padded plane allocation (max read 1151 + 34 < 1216)
PLANE_ELEMS = FK * PLANE_STRIDE

# stage-4 spatial blocking: 4 output rows per matmul -> N = 384
S4_ROWS = 4
S4_N = S4_ROWS * W  # 384
S4_BLOCKS = H // S4_ROWS  # 24


def _sub(t, base, dims):
    """Strided sub-view of a 2D SBUF tile: keep partition dim, free dims =
    [[step, count], ...] starting at element offset `base`."""
    return bass.AP(tensor=t.tensor, offset=t.offset + base,
                   ap=[list(t.ap[0])] + [list(d) for d in dims])


def _emit(nc, tc, tens):
    from contextlib import ExitStack
    ctx = ExitStack()
    # right-stack pools, LIFO: x1p+raw outlive kq outlive r.
    cm_xp = tc.tile_pool(name="pool_xp", bufs=1, side="right")
    pool_xp = cm_xp.__enter__()
    cm_kq = tc.tile_pool(name="pool_kq", bufs=1, side="right")
    pool_kq = cm_kq.__enter__()
    cm_r = tc.tile_pool(name="pool_r", bufs=1, side="right")
    pool_r = cm_r.__enter__()
    const = ctx.enter_context(tc.tile_pool(name="const", bufs=1))
    work = ctx.enter_context(tc.tile_pool(name="work", bufs=1))
    psum = ctx.enter_context(tc.tile_pool(name="psum", bufs=6, space="PSUM"))
    pstat = ctx.enter_context(tc.tile_pool(name="pstat", bufs=2, space="PSUM"))

    ccols = (C0, C1)

    # ---- load inputs to SBUF ----------------------------------------------
    # chunk-1 (64-partition) tensors are packed pairwise onto 128 partitions:
    #   sr1: parts 0:64 = x2r chunk1, parts 64:128 = x1r chunk1
    #   sw1: parts 0:64 = wq chunk1,  parts 64:128 = wk chunk1
    # (matmul requires lhsT/rhs to share base_partition, so the x chunk and
    # its matching weight chunk sit at the same base.)
    sx1r0 = pool_r.tile([C0, PLANE_ELEMS], BF16, tag="sx1r0")
    sx2r0 = pool_r.tile([C0, PLANE_ELEMS], BF16, tag="sx2r0")
    sr1 = pool_r.tile([128, PLANE_ELEMS], BF16, tag="sr1")
    swk0 = pool_r.tile([C0, FK * COUT], BF16, tag="swk0")
    swq0 = pool_r.tile([C0, FK * COUT], BF16, tag="swq0")
    sw1 = pool_r.tile([128, FK * COUT], BF16, tag="sw1")
    sx1p = [pool_xp.tile([ccols[i], NPP], BF16, tag=f"sx1p_{i}",
                         name=f"sx1p_{i}") for i in range(2)]
    skm = const.tile([1, 1], F32, tag="skm")
    smask = const.tile([128, NCHK], F32, tag="smask")
    nc.sync.dma_start(swk0[:], tens["wk0"][:])
    nc.sync.dma_start(swq0[:], tens["wq0"][:])
    nc.sync.dma_start(sw1[64:128, :], tens["wk1"][:])
    nc.sync.dma_start(sw1[0:64, :], tens["wq1"][:])
    nc.sync.dma_start(skm[:], tens["km"][:])
    nc.sync.dma_start(smask[:], tens["msk"][:])
    nc.sync.dma_start(sx1r0[:], tens["x1r0"][:])
    nc.sync.dma_start(sr1[64:128, :], tens["x1r1"][:])
    nc.sync.dma_start(sx2r0[:], tens["x2r0"][:])
    nc.sync.dma_start(sr1[0:64, :], tens["x2r1"][:])
    for i in range(2):
        nc.sync.dma_start(sx1p[i][:], tens[f"x1p{i}"][:])

    sx1r = (sx1r0, sr1[64:128, :])
    sx2r = (sx2r0, sr1[0:64, :])
    swk = (swk0, sw1[64:128, :])
    swq = (swq0, sw1[0:64, :])

    # warm-up AllReduce: the first collective call pays a one-time setup
    # cost; fire a dummy one immediately so it overlaps the convs.
    cc_win = nc.dram_tensor("cc_win", [1, 2], F32)
    cc_wout = nc.dram_tensor("cc_wout", [1, 2], F32, addr_space="Shared")
    zz = work.tile([1, 2], F32, tag="zz")
    nc.vector.memset(zz[:], 0.0)
    nc.sync.dma_start(cc_win[:], zz[:])
    nc.gpsimd.collective_compute(
        "AllReduce", mybir.AluOpType.add,
        replica_groups=[list(range(N_CORES))],
        ins=[cc_win[:]], outs=[cc_wout[:]],
    )

    # ---- stage 1+2: key / query convs, transposed orientation -------------
    # dst[pix, (k*NCHK + c)*COUT + o]; pixel chunk c = flat patch indices
    # [c*128, c*128+128) on the 33-wide grid of class k
    keyT = pool_kq.tile([128, FK * NCHK * COUT], BF16, tag="keyT")
    queryT = pool_kq.tile([128, FK * NCHK * COUT], BF16, tag="queryT")
    attn = [const.tile([ccols[i], FK * COUT], F32, tag=f"attn_{i}",
                       name=f"attn_{i}") for i in range(2)]

    def conv_t(xin, win, dst, mask, post_class=None):
        for kh in range(KS):
            for kw in range(KS):
                k = kh * KS + kw
                for c in range(NCHK):
                    pt = psum.tile([128, COUT], F32, tag="mm", name="pt")
                    i_mm = 0
                    for cc in range(2):
                        for dy in range(KS):
                            for dx in range(KS):
                                off = dy * KS + dx
                                kp = ((kh + dy) % 3) * 3 + ((kw + dx) % 3)
                                sh = ((kh + dy) // 3) * PQ + ((kw + dx) // 3)
                                base = kp * PLANE_STRIDE + c * 128 + sh
                                lhsT = _sub(xin[cc], base, [[1, 128]])
                                rhs = win[cc][:, off * COUT:(off + 1) * COUT]
                                nc.tensor.matmul(
                                    pt[:], lhsT, rhs,
                                    start=(i_mm == 0), stop=(i_mm == 17))
                                i_mm += 1
                    col = (k * NCHK + c) * COUT
                    if mask:
                        nc.scalar.activation(
                            dst[:, col:col + COUT], pt[:],
                            mybir.ActivationFunctionType.Copy,
                            scale=smask[:, c:c + 1])
                    else:
                        nc.any.tensor_copy(dst[:, col:col + COUT], pt[:])
                if post_class is not None:
                    post_class(k)

    def attn_class(k):
        # attn[k, ci, co] = sum over the 9 pixel chunks of class k
        for cc in range(2):
            pa = psum.tile([ccols[cc], COUT], F32, tag="mm", name="pa")
            for c in range(NCHK):
                col = (k * NCHK + c) * COUT
                lo = cc * C0
                lhsT = keyT[:, col + lo:col + lo + ccols[cc]]
                rhs = queryT[:, col:col + COUT]
                nc.tensor.matmul(pa[:], lhsT, rhs,
                                 start=(c == 0), stop=(c == NCHK - 1))
            nc.any.tensor_copy(attn[cc][:, k * COUT:(k + 1) * COUT], pa[:])

    conv_t(sx1r, swk, keyT, mask=True)
    conv_t(sx2r, swq, queryT, mask=False, post_class=attn_class)
    cm_r.__exit__(None, None, None)
    cm_kq.__exit__(None, None, None)
    pool_raw = tc.tile_pool(name="pool_raw", bufs=1, side="right")
    praw = pool_raw.__enter__()

    # ---- global stats: sum / sumsq -> AllReduce ---------------------------
    # (the attnh raw-cast below doubles as the square scratch target)
    attnh = [const.tile([ccols[i], FK * COUT], BF16, tag=f"attnh_{i}",
                        name=f"attnh_{i}") for i in range(2)]
    stats = work.tile([128, 2], F32, tag="stats")
    sums1 = work.tile([C1, 1], F32, tag="sums1")
    sq0 = work.tile([128, 1], F32, tag="sq0")
    sq1 = work.tile([C1, 1], F32, tag="sq1")
    nc.vector.tensor_reduce(stats[:, 0:1], attn[0][:], mybir.AxisListType.X,
                            mybir.AluOpType.add)
    nc.vector.tensor_reduce(sums1[:], attn[1][:], mybir.AxisListType.X,
                            mybir.AluOpType.add)
    nc.scalar.activation(attnh[0][:, :], attn[0][:],
                         mybir.ActivationFunctionType.Square, accum_out=sq0[:])
    nc.scalar.activation(attnh[1][:, :], attn[1][:],
                         mybir.ActivationFunctionType.Square, accum_out=sq1[:])
    nc.vector.tensor_copy(stats[:, 1:2], sq0[:])
    nc.vector.tensor_add(stats[:C1, 0:1], stats[:C1, 0:1], sums1[:])
    nc.vector.tensor_add(stats[:C1, 1:2], stats[:C1, 1:2], sq1[:])

    ones_col = const.tile([128, 1], F32, tag="ones_col")
    nc.vector.memset(ones_col[:], 1.0)
    ps_red = pstat.tile([1, 2], F32, tag="pstat", name="ps_red",
                        padded_shape=[128, 2])
    nc.tensor.matmul(ps_red[:], ones_col[:], stats[:], start=True, stop=True)
    sred = work.tile([1, 2], F32, tag="sred")
    nc.any.tensor_copy(sred[:], ps_red[:])

    cc_in = nc.dram_tensor("cc_in", [1, 2], F32)
    cc_out = nc.dram_tensor("cc_out", [1, 2], F32, addr_space="Shared")
    nc.sync.dma_start(cc_in[:], sred[:])
    nc.gpsimd.collective_compute(
        "AllReduce", mybir.AluOpType.add,
        replica_groups=[list(range(N_CORES))],
        ins=[cc_in[:]], outs=[cc_out[:]],
    )
    gred = work.tile([1, 2], F32, tag="gred")
    nc.sync.dma_start(gred[:], cc_out[:])

    # ---- raw-cast attn to bf16 (normalization deferred past stage 4) ------
    # attnh[cc][ci, k*192+o] = attn (raw); attnh_e[cc][ci, k*65 + (0..63)] =
    # attn cols 128..191 with a ones column at k*65+64 (computes vsum row).
    attnh_e = [const.tile([ccols[i], FK * 65], BF16, tag=f"attnhe_{i}",
                          name=f"attnhe_{i}") for i in range(2)]
    for cc in range(2):
        nc.scalar.copy(attnh[cc][:], attn[cc][:])
        nc.scalar.activation(
            _sub(attnh_e[cc], 0, [[65, FK], [1, 64]]),
            _sub(attn[cc], C0, [[COUT, FK], [1, 64]]),
            mybir.ActivationFunctionType.Copy)
        nc.vector.memset(_sub(attnh_e[cc], 64, [[65, FK], [1, 1]]), 1.0)

    # ---- stage 4: raw = conv3x3(x1, attn_raw), standard orientation -------
    # oc=1 uses the extended stationary (M=65); psum row 64 = vsum block.
    ocols_e = (C0, C1 + 1)
    raws = [[praw.tile([ocols_e[oc], S4_N], F32, tag=f"raw{oc}_{blk}",
                       name=f"raw{oc}_{blk}") for oc in range(2)]
            for blk in range(S4_BLOCKS)]
    for blk in range(S4_BLOCKS):
        for oc in range(2):
            po = psum.tile([ocols_e[oc], S4_N], F32, tag="mm", name="po")
            i_mm = 0
            for cc in range(2):
                for kh in range(KS):
                    for kw in range(KS):
                        k = kh * KS + kw
                        if oc == 0:
                            lhsT = attnh[cc][:, k * COUT:k * COUT + C0]
                        else:
                            lhsT = attnh_e[cc][:, k * 65:(k + 1) * 65]
                        base = (S4_ROWS * blk + kh) * HP + kw
                        rhs = _sub(sx1p[cc], base, [[HP, S4_ROWS], [1, W]])
                        nc.tensor.matmul(po[:], lhsT, rhs,
                                         start=(i_mm == 0), stop=(i_mm == 17))
                        i_mm += 1
            nc.scalar.copy(raws[blk][oc][:], po[:])

    # ---- scalar math on DVE only: alpha = km + 1/(s+eps), beta = -m/(s+eps)
    # (DVE has no other pending work, so its in-order stream can safely park
    # on the collective result without head-of-line blocking anything.)
    sc = work.tile([1, 8], F32, tag="sc")
    m_ = sc[:, 0:1]; t1 = sc[:, 1:2]; t2 = sc[:, 2:3]; sd = sc[:, 3:4]
    r_ = sc[:, 4:5]; al = sc[:, 5:6]; be = sc[:, 6:7]
    S_ = gred[:, 0:1]; SS = gred[:, 1:2]
    nc.vector.tensor_scalar_mul(m_, S_, 1.0 / NTOT)
    nc.vector.tensor_mul(t1, S_, m_)
    nc.vector.tensor_sub(t2, SS, t1)
    nc.vector.tensor_scalar_mul(t2, t2, 1.0 / (NTOT - 1.0))
    # sqrt on DVE: rsqrt via exponent bit-trick + 3 Newton steps (fp32-exact),
    # then s = var * rsqrt(var). Keeps the whole chain off the ACT engine.
    I32 = mybir.dt.int32
    sc2 = work.tile([1, 4], F32, tag="sc2")
    hv = sc2[:, 0:1]; y_ = sc2[:, 1:2]; tn = sc2[:, 2:3]
    magic = work.tile([1, 1], I32, tag="magic")
    nc.vector.memset(magic[:], 0x5F3759DF)
    nc.vector.tensor_scalar_mul(hv, t2, 0.5)
    nc.vector.tensor_scalar(y_.bitcast(I32), t2.bitcast(I32), 1, None,
                            op0=mybir.AluOpType.logical_shift_right)
    nc.vector.tensor_sub(y_.bitcast(I32), magic[:], y_.bitcast(I32))
    for _ in range(3):
        nc.vector.tensor_mul(tn, y_, y_)
        nc.vector.tensor_mul(tn, tn, hv)
        nc.vector.tensor_scalar(tn, tn, -1.0, 1.5,
                                op0=mybir.AluOpType.mult,
                                op1=mybir.AluOpType.add)
        nc.vector.tensor_mul(y_, y_, tn)
    nc.vector.tensor_mul(sd, t2, y_)
    nc.vector.tensor_scalar_add(sd, sd, EPS)
    nc.vector.reciprocal(r_, sd)
    nc.vector.tensor_add(al, r_, skm[:])
    nc.vector.tensor_mul(be, m_, r_)
    nc.vector.tensor_scalar_mul(be, be, -1.0)

    ab1 = work.tile([1, 2], F32, tag="ab1")
    nc.vector.tensor_copy(ab1[:, 0:1], al)
    nc.vector.tensor_copy(ab1[:, 1:2], be)
    ab = work.tile([128, 2], F32, tag="ab")
    nc.gpsimd.partition_broadcast(ab[:], ab1[:])

    # ---- fixup: out = alpha*raw + beta*vsum, then DMA out -----------------
    out = tens["out"]
    ocols = (C0, C1)
    # beta*vsum in chunks of 4 blocks: DVE computes the row, gpsimd
    # broadcasts across partitions, DVE applies out = alpha*raw + bvs.
    FBPC = 4
    for q in range(S4_BLOCKS // FBPC):
        bvs = praw.tile([1, FBPC * S4_N], F32, tag="bvs", name="bvs", bufs=2)
        for j in range(FBPC):
            blk = q * FBPC + j
            nc.vector.tensor_scalar_mul(
                bvs[:, j * S4_N:(j + 1) * S4_N], raws[blk][1][64:65, :], be)
        pb = praw.tile([128, FBPC * S4_N], F32, tag="pb", name="pb", bufs=2)
        nc.gpsimd.partition_broadcast(pb[:], bvs[:])
        for j in range(FBPC):
            blk = q * FBPC + j
            for oc in range(2):
                obf = praw.tile([ocols[oc], S4_N], F32, tag=f"obf_{oc}",
                                name=f"obf_{oc}", bufs=2)
                nc.vector.scalar_tensor_tensor(
                    obf[:], raws[blk][oc][:ocols[oc], :],
                    ab[:ocols[oc], 0:1],
                    pb[:ocols[oc], j * S4_N:(j + 1) * S4_N],
                    op0=mybir.AluOpType.mult, op1=mybir.AluOpType.add)
                nc.sync.dma_start(
                    out[oc * C0:oc * C0 + ocols[oc],
                        blk * S4_N:(blk + 1) * S4_N],
                    obf[:])
    pool_raw.__exit__(None, None, None)
    cm_xp.__exit__(None, None, None)
    ctx.close()


def build_nc():
    nc = bacc.Bacc("TRN2", target_bir_lowering=False, debug=False,
                   num_devices=N_CORES)
    tens = {}
    for i, cc in enumerate((C0, C1)):
        tens[f"x1r{i}"] = nc.dram_tensor(f"x1r{i}", [cc, PLANE_ELEMS], BF16,
                                         kind="ExternalInput")
        tens[f"x2r{i}"] = nc.dram_tensor(f"x2r{i}", [cc, PLANE_ELEMS], BF16,
                                         kind="ExternalInput")
        tens[f"x1p{i}"] = nc.dram_tensor(f"x1p{i}", [cc, NPP], BF16,
                                         kind="ExternalInput")
        tens[f"wk{i}"] = nc.dram_tensor(f"wk{i}", [cc, FK * COUT], BF16,
                                        kind="ExternalInput")
        tens[f"wq{i}"] = nc.dram_tensor(f"wq{i}", [cc, FK * COUT], BF16,
                                        kind="ExternalInput")
    tens["km"] = nc.dram_tensor("km", [1, 1], F32, kind="ExternalInput")
    tens["msk"] = nc.dram_tensor("msk", [128, NCHK], F32, kind="ExternalInput")
    tens["out"] = nc.dram_tensor("out", [COUT, HW], F32, kind="ExternalOutput")
    with tile.TileContext(nc) as tc:
        _emit(nc, tc, tens)
    nc.finalize()
    return nc


_NC = None
LAST_RESULTS = None  # BassKernelResults of the most recent run (for test.py)


def _prep_image_planes(x):
    """[192, 96, 96] fp32 -> residue-class planes [192, 9*PLANE_STRIDE] bf16.
    Plane (r,s)[p,q] = xpad[3p+r, 3q+s] on the zero-padded (98x98) image."""
    xp = np.zeros((CIN, 99, 99), dtype=np.float32)
    xp[:, 1:1 + H, 1:1 + W] = x
    v = xp.reshape(CIN, PQ, 3, PQ, 3).transpose(0, 2, 4, 1, 3)
    v = v.reshape(CIN, FK, PQ * PQ).astype(ml_dtypes.bfloat16)
    out = np.zeros((CIN, FK, PLANE_STRIDE), dtype=ml_dtypes.bfloat16)
    out[:, :, :PQ * PQ] = v
    return out.reshape(CIN, PLANE_ELEMS)


def _prep_image_pad(x):
    """[192, 96, 96] fp32 -> padded [192, 98*98] bf16."""
    xp = np.zeros((CIN, HP, HP), dtype=ml_dtypes.bfloat16)
    xp[:, 1:1 + H, 1:1 + W] = x.astype(ml_dtypes.bfloat16)
    return xp.reshape(CIN, NPP)


def _prep_w(w):
    """[O, I, 3, 3] fp32 -> ([128, 9*192], [64, 9*192]) bf16, [ci, off*192+o]."""
    wt = np.ascontiguousarray(w.transpose(1, 2, 3, 0)).reshape(CIN, FK * COUT)
    wt = wt.astype(ml_dtypes.bfloat16)
    return wt[:C0], wt[C0:]


def _chunk_mask():
    f = np.arange(NCHK * 128).reshape(NCHK, 128)
    valid = (f < PLANE_VALID) & (f % PQ < 32)
    return np.ascontiguousarray(valid.T).astype(np.float32)


def make_in_maps(x1, x2, key_w, query_w, kernel_momentum):
    x1 = np.asarray(x1, dtype=np.float32)
    x2 = np.asarray(x2, dtype=np.float32)
    key_w = np.asarray(key_w, dtype=np.float32)
    query_w = np.asarray(query_w, dtype=np.float32)
    km = np.asarray(kernel_momentum, dtype=np.float32).reshape(1, 1)
    wk0, wk1 = _prep_w(key_w)
    wq0, wq1 = _prep_w(query_w)
    msk = _chunk_mask()
    in_maps = []
    for b in range(N_CORES):
        xr1 = _prep_image_planes(x1[b])
        xr2 = _prep_image_planes(x2[b])
        xp1 = _prep_image_pad(x1[b])
        in_maps.append({
            "x1r0": xr1[:C0], "x1r1": xr1[C0:],
            "x2r0": xr2[:C0], "x2r1": xr2[C0:],
            "x1p0": xp1[:C0], "x1p1": xp1[C0:],
            "wk0": wk0, "wk1": wk1, "wq0": wq0, "wq1": wq1,
            "km": km, "msk": msk,
        })
    return in_maps


def kernel(x1, x2, key_w, query_w, kernel_momentum):
    global _NC, LAST_RESULTS
    if _NC is None:
        _NC = build_nc()
    in_maps = make_in_maps(x1, x2, key_w, query_w, kernel_momentum)
    trace = bool(int(os.environ.get("BASS_KERNEL_TRACE", "0")))
    res = run_bass_kernel_spmd(_NC, in_maps, list(range(N_CORES)), trace=trace)
    LAST_RESULTS = res
    out = np.stack([res.results[b]["out"].reshape(COUT, H, W)
                    for b in range(N_CORES)])
    return out.astype(np.float32)
